# revision 6
# baseline (speedup 1.0000x reference)
"""Trainium2 Bass kernel for masked-mean-pooling + per-attribute softmax/loss.

Data-parallel over the batch: 8 NeuronCores x 2048 users each.

Gather strategy: dma_gather is the only high-rate gather primitive but takes
int16 indices, so the table is scattered (fixed pseudo-random permutation)
into 4 equal chunks of 25024 rows. Per user the indices are deduplicated and
carried with multiplicity weights; the randomized chunk mapping keeps the
per-user per-chunk distinct counts near Binomial(d, 1/4), so compile-time
per-tile window widths (users sorted by their max per-chunk count) fit with
high probability. Unused window slots point at a zero table row with weight
zero. Three window profiles (tight/medium/safe) are compiled on demand and
picked per input by an exact host-side fit check, so the kernel is never
wrong, only slower on unusual inputs.

Each 1024-index gather lands in its own SBUF tile (single-packet mode, 4
SWDGE queues round-robin); a DVE multiply applies the multiplicity weights
into a per-tile accumulation buffer, one strided reduce sums the history
dim, the mask-length column is appended, PE transposes [128,65]->[65,128]
and a matmul against the host-packed [65,16] weights (bias folded in via the
length column) produces the logits' pre-activations. Epilogue: scale by
1/len, per-attribute softmax + log-softmax, per-core loss partials reduced
across partitions with a ones-matmul; the host sums partials into the
scalar loss.
"""

import os
import sys

if "/opt/trn_rl_repo" not in sys.path:
    sys.path.insert(0, "/opt/trn_rl_repo")

import numpy as np

N_CORES = 8
B, L, V, D = 16384, 50, 100000, 64
BL = B // N_CORES          # 2048 users per core
P = 128                    # partitions
NT = BL // P               # 16 tiles per core
ATTR = (2, 6, 8)
NA = 16

NCHUNK = 4
CHN = 25024                # rows per chunk (int16-addressable)
DUMMY = CHN                # local index of the zero row in each chunk block
VDEV = NCHUNK * (CHN + 1)

# fixed pseudo-random table permutation: original row -> (chunk, local)
_tp_pos = np.random.default_rng(12345).permutation(NCHUNK * CHN)
TP_CHUNK = (_tp_pos // CHN).astype(np.int32)       # [100096]
TP_LOCAL = (_tp_pos % CHN).astype(np.int32)

# per-tile window widths (applied to each of the 4 chunks)
PROFILE_T = (28, 19, 18, 17, 16, 15, 14, 13, 12, 12, 11, 10, 8, 8, 7, 5)
PROFILE_M = (29, 22, 21, 21, 20, 20, 20, 19, 19, 19, 18, 18, 18, 18, 17, 17)
PROFILE_S = (50,) * 16

_CACHE = {}


def _build_nc(profile):
    import concourse.bass as bass
    import concourse.tile as tile
    from concourse import mybir
    from concourse.bacc import Bacc
    from concourse.masks import make_identity

    f32 = mybir.dt.float32
    i16 = mybir.dt.int16
    Alu = mybir.AluOpType
    Ax = mybir.AxisListType
    Act = mybir.ActivationFunctionType

    tot_cols = [NCHUNK * w for w in profile]
    tot = sum(tot_cols)
    idx_free = 8 * tot

    nc = Bacc(None, target_bir_lowering=False, num_swdge_queues=4)

    idx_d = nc.dram_tensor("idx", [16, idx_free], i16, kind="ExternalInput")
    wg_d = nc.dram_tensor("wg", [P, tot], f32, kind="ExternalInput")
    mask_d = nc.dram_tensor("mask", [BL, L], f32, kind="ExternalInput")
    y_d = nc.dram_tensor("y", [BL, NA], f32, kind="ExternalInput")
    ob_d = nc.dram_tensor("ob", [BL, NA], f32, kind="ExternalInput")
    emb_d = nc.dram_tensor("emb", [VDEV, D], f32, kind="ExternalInput")
    wbT_d = nc.dram_tensor("wbT", [D + 1, NA], f32, kind="ExternalInput")
    logit_d = nc.dram_tensor("logit", [BL, NA], f32, kind="ExternalOutput")
    part_d = nc.dram_tensor("partials", [1, 8], f32, kind="ExternalOutput")

    acc_bufs = 1 if max(profile) >= 40 else 3

    with tile.TileContext(nc) as tc:
        with (
            tc.tile_pool(name="const", bufs=1) as constp,
            tc.tile_pool(name="gsub", bufs=12) as gsubp,
            tc.tile_pool(name="acc", bufs=acc_bufs) as accp,
            tc.tile_pool(name="work", bufs=3) as workp,
            tc.tile_pool(name="epil", bufs=1) as epil,
            tc.tile_pool(name="pst", bufs=2, space="PSUM") as pst,
            tc.tile_pool(name="psw", bufs=1, space="PSUM") as psw,
        ):
            identity = constp.tile([P, P], f32)
            make_identity(nc, identity[:])

            wbT = constp.tile([D + 1, NA], f32)
            nc.sync.dma_start(out=wbT[:], in_=wbT_d[:])

            idx_sb = constp.tile([P, idx_free], i16)
            # split the replicated load so the first gathers start early
            splits = [0, 8 * tot_cols[0], idx_free]
            for a, b in zip(splits[:-1], splits[1:]):
                src = bass.AP(tensor=idx_d[:].tensor, offset=a,
                              ap=[[0, 8], [idx_free, 16], [1, b - a]])
                nc.sync.dma_start(out=idx_sb[:, a:b], in_=src)

            wg_sb = constp.tile([P, tot, 1], f32)
            nc.sync.dma_start(out=wg_sb[:], in_=wg_d[:].rearrange("p t -> p t ()"))

            mask_sb = constp.tile([P, NT, L], f32)
            nc.sync.dma_start(
                out=mask_sb[:], in_=mask_d.rearrange("(t p) l -> p t l", p=P)
            )
            y_sb = constp.tile([P, NT, NA], f32)
            nc.sync.dma_start(out=y_sb[:], in_=y_d.rearrange("(t p) j -> p t j", p=P))
            ob_sb = constp.tile([P, NT, NA], f32)
            nc.sync.dma_start(out=ob_sb[:], in_=ob_d.rearrange("(t p) j -> p t j", p=P))

            ones = constp.tile([P, 1], f32)
            nc.vector.memset(ones[:], 1.0)

            lens = epil.tile([P, NT, 1], f32)
            rlen = epil.tile([P, NT, 1], f32)
            W_ps = psw.tile([P, NT, NA], f32)

            qn = 0
            off = 0   # column offset into the global stream
            for t in range(NT):
                wt = profile[t]
                tcols = tot_cols[t]
                embacc = accp.tile([P, tcols, D], f32, tag="acc")
                cbase = 0
                for k in range(NCHUNK):
                    emb_view = emb_d[k * (CHN + 1):(k + 1) * (CHN + 1), :]
                    c0 = 0
                    while c0 < wt:
                        w = min(8, wt - c0)
                        ni = w * P
                        col = off + cbase + c0
                        gsub = gsubp.tile([P, 8, D], f32, tag="g")
                        nc.gpsimd.dma_gather(
                            out_ap=gsub[:, 0:w, :],
                            in_ap=emb_view,
                            idxs_ap=idx_sb[:, col * 8: (col + w) * 8],
                            num_idxs=ni,
                            num_idxs_reg=ni,
                            elem_size=D,
                            single_packet=True,
                            queue_num=qn,
                        )
                        qn = (qn + 1) % 4
                        nc.vector.tensor_tensor(
                            out=embacc[:, cbase + c0: cbase + c0 + w, :],
                            in0=gsub[:, 0:w, :],
                            in1=wg_sb[:, col: col + w, :].to_broadcast([P, w, D]),
                            op=Alu.mult,
                        )
                        c0 += w
                    cbase += wt
                off += tcols

                s65 = workp.tile([P, D + 1], f32)
                nc.vector.tensor_reduce(
                    out=s65[:, 0:D],
                    in_=embacc[:].rearrange("p c d -> p d c"),
                    axis=Ax.X,
                    op=Alu.add,
                )
                nc.vector.tensor_reduce(
                    out=lens[:, t, :], in_=mask_sb[:, t, :], axis=Ax.X, op=Alu.add
                )
                nc.vector.tensor_copy(out=s65[:, D: D + 1], in_=lens[:, t, :])
                tp = pst.tile([D + 1, P], f32)
                nc.tensor.transpose(out=tp[:], in_=s65[:], identity=identity[:])
                urT = workp.tile([D + 1, P], f32)
                nc.scalar.copy(out=urT[:], in_=tp[:])
                nc.tensor.matmul(W_ps[:, t, :], urT[:], wbT[:], start=True, stop=True)

            # ---- epilogue over all 2048 users: [P, NT, 16] ----
            nc.vector.reciprocal(out=rlen[:], in_=lens[:])
            W = epil.tile([P, NT, NA], f32)
            nc.vector.tensor_tensor(
                out=W[:], in0=W_ps[:], in1=rlen[:].to_broadcast([P, NT, NA]),
                op=Alu.mult,
            )

            S = epil.tile([P, NT, NA], f32)
            E = epil.tile([P, NT, NA], f32)
            LG = epil.tile([P, NT, NA], f32)
            yob = epil.tile([P, NT, NA], f32)
            nc.vector.tensor_tensor(out=yob[:], in0=y_sb[:], in1=ob_sb[:], op=Alu.mult)
            part = epil.tile([P, 8], f32)
            nc.vector.memset(part[:], 0.0)

            s = 0
            for g, w in enumerate(ATTR):
                sl = slice(s, s + w)
                mx = epil.tile([P, NT, 1], f32, tag=f"mx{g}")
                nc.vector.tensor_reduce(
                    out=mx[:], in_=W[:, :, sl], axis=Ax.X, op=Alu.max
                )
                nc.vector.tensor_tensor(
                    out=S[:, :, sl], in0=W[:, :, sl],
                    in1=mx[:].to_broadcast([P, NT, w]), op=Alu.subtract,
                )
                s += w

            nc.scalar.activation(out=E[:], in_=S[:], func=Act.Exp)

            s = 0
            for g, w in enumerate(ATTR):
                sl = slice(s, s + w)
                se = epil.tile([P, NT, 1], f32, tag=f"se{g}")
                nc.vector.tensor_reduce(
                    out=se[:], in_=E[:, :, sl], axis=Ax.X, op=Alu.add
                )
                rse = epil.tile([P, NT, 1], f32, tag=f"rse{g}")
                nc.vector.reciprocal(out=rse[:], in_=se[:])
                nc.vector.tensor_tensor(
                    out=LG[:, :, sl], in0=E[:, :, sl],
                    in1=rse[:].to_broadcast([P, NT, w]), op=Alu.mult,
                )
                lse = epil.tile([P, NT, 1], f32, tag=f"lse{g}")
                nc.scalar.activation(out=lse[:], in_=se[:], func=Act.Ln)
                nc.vector.tensor_tensor(
                    out=S[:, :, sl], in0=S[:, :, sl],
                    in1=lse[:].to_broadcast([P, NT, w]), op=Alu.subtract,
                )
                C = epil.tile([P, NT, w], f32, tag=f"C{g}")
                nc.vector.tensor_tensor(
                    out=C[:], in0=yob[:, :, sl], in1=S[:, :, sl], op=Alu.mult
                )
                nc.vector.tensor_reduce(
                    out=part[:, g: g + 1], in_=C[:], axis=Ax.XY, op=Alu.add
                )
                om = epil.tile([P, NT, 1], f32, tag=f"om{g}")
                nc.vector.tensor_reduce(
                    out=om[:], in_=ob_sb[:, :, sl], axis=Ax.X, op=Alu.max
                )
                nc.vector.tensor_reduce(
                    out=part[:, 4 + g: 5 + g], in_=om[:], axis=Ax.XY, op=Alu.add
                )
                s += w

            red = pst.tile([1, 8], f32, tag="red")
            nc.tensor.matmul(red[:], ones[:], part[:], start=True, stop=True)
            out_sb = epil.tile([1, 8], f32)
            nc.scalar.copy(out=out_sb[:], in_=red[:])
            nc.sync.dma_start(out=part_d[:], in_=out_sb[:])
            nc.sync.dma_start(
                out=logit_d.rearrange("(t p) j -> p t j", p=P), in_=LG[:]
            )

    nc.compile()
    return nc


def _get_nc(profile):
    if profile not in _CACHE:
        _CACHE[profile] = _build_nc(profile)
    return _CACHE[profile]


def _analyze_core(xc):
    """Dedup + chunk stats for one core. Returns dict of arrays."""
    xs = np.sort(xc, axis=1)                      # [BL, L]
    fo = np.concatenate(
        [np.ones((BL, 1), bool), xs[:, 1:] != xs[:, :-1]], axis=1
    )
    wgt = (xs[:, :, None] == xs[:, None, :]).sum(axis=2).astype(np.float32)
    ch = TP_CHUNK[xs]
    loc = TP_LOCAL[xs]
    n = np.stack([((ch == k) & fo).sum(axis=1) for k in range(NCHUNK)], axis=1)
    return dict(xs=xs, fo=fo, wgt=wgt, ch=ch, loc=loc, n=n)


def _fits(an, profile):
    nmax_sorted = np.sort(an["n"].max(axis=1))[::-1]
    wt = np.repeat(np.asarray(profile), P)
    return (nmax_sorted <= wt).all()


def _pack_core(an, profile):
    """Build slot permutation, idx stream and weight stream for one core."""
    nmax = an["n"].max(axis=1)
    order = np.argsort(-nmax, kind="stable")      # slot s -> original user
    fo, ch, loc, wgt = an["fo"], an["ch"], an["loc"], an["wgt"]

    idx_blocks = []
    wg_blocks = []
    for t in range(NT):
        wt = profile[t]
        users = order[t * P:(t + 1) * P]
        kt = ch[users]
        lt = loc[users]
        ft = fo[users]
        gt = wgt[users]
        for kk in range(NCHUNK):
            m = ft & (kt == kk)
            cc = np.cumsum(m, axis=1) - 1
            A = np.full((P, wt), DUMMY, dtype=np.int16)
            WG = np.zeros((P, wt), dtype=np.float32)
            rows, cols = np.where(m)
            A[rows, cc[rows, cols]] = lt[rows, cols].astype(np.int16)
            WG[rows, cc[rows, cols]] = gt[rows, cols]
            flat = A.T.ravel()                    # stream pos i = c*128 + p
            idx_blocks.append(flat.reshape(-1, 16).T)
            wg_blocks.append(WG)
    idx_wrapped = np.ascontiguousarray(np.concatenate(idx_blocks, axis=1))
    wg_host = np.ascontiguousarray(np.concatenate(wg_blocks, axis=1))
    return order, idx_wrapped, wg_host


def _install_trace_shim():
    import types

    import antenv

    if "antenv.axon_hooks" in sys.modules:
        return
    mod = types.ModuleType("antenv.axon_hooks")
    hook = [None]
    mod.set_axon_ntff_profile_hook = lambda h: hook.__setitem__(0, h)
    mod.get_axon_ntff_profile_hook = lambda: hook[0]
    sys.modules["antenv.axon_hooks"] = mod
    antenv.axon_hooks = mod
    if "/root/.axon_site" not in sys.path:
        sys.path.insert(0, "/root/.axon_site")
    try:
        from trn_agent_boot.trn_boot import _ntff_profile_via_ctypes

        mod.set_axon_ntff_profile_hook(
            _ntff_profile_via_ctypes("/opt/axon/libaxon_pjrt.so")
        )
    except Exception:
        pass


def kernel(**inputs):
    from concourse.bass_utils import run_bass_kernel_spmd

    x = np.asarray(inputs["x"]).astype(np.int32).reshape(N_CORES, BL, L)
    mask = np.asarray(inputs["x_mask"]).astype(np.float32).reshape(N_CORES, BL, L)
    y = np.asarray(inputs["y"]).astype(np.float32).reshape(N_CORES, BL, NA)
    ob = np.asarray(inputs["ob"]).astype(np.float32).reshape(N_CORES, BL, NA)
    emb = np.asarray(inputs["item_emb"], dtype=np.float32)

    # permuted device table: 4 blocks of CHN+1 rows, last row of each is zero
    emb_dev = np.zeros((VDEV, D), dtype=np.float32)
    rows = np.arange(V)
    emb_dev[TP_CHUNK[:V] * (CHN + 1) + TP_LOCAL[:V]] = emb[rows]
    emb_dev = np.ascontiguousarray(emb_dev)

    wcat = np.concatenate(
        [np.asarray(inputs[f"w{i}"], dtype=np.float32) for i in range(3)], axis=0
    )
    bcat = np.concatenate(
        [np.asarray(inputs[f"b{i}"], dtype=np.float32) for i in range(3)], axis=0
    )
    wbT = np.ascontiguousarray(
        np.concatenate([wcat.T, bcat[None, :]], axis=0).astype(np.float32)
    )

    analyses = [_analyze_core(x[c]) for c in range(N_CORES)]
    profile = None
    for prof in (PROFILE_T, PROFILE_M, PROFILE_S):
        if all(_fits(an, prof) for an in analyses):
            profile = prof
            break
    assert profile is not None  # PROFILE_S always fits (counts <= 50)

    in_maps = []
    perms = []
    for c in range(N_CORES):
        perm, idxw, wgh = _pack_core(analyses[c], profile)
        perms.append(perm)
        in_maps.append({
            "idx": idxw,
            "wg": wgh,
            "mask": np.ascontiguousarray(mask[c][perm]),
            "y": np.ascontiguousarray(y[c][perm]),
            "ob": np.ascontiguousarray(ob[c][perm]),
            "emb": emb_dev,
            "wbT": wbT,
        })

    nc = _get_nc(profile)
    trace = os.environ.get("KERNEL_TRACE") == "1"
    if trace:
        _install_trace_shim()
    res = run_bass_kernel_spmd(
        nc, in_maps, core_ids=list(range(N_CORES)), trace=trace
    )
    if trace:
        _CACHE["exec_time_ns"] = res.exec_time_ns
        _CACHE["profile_json"] = res.profile_json

    logit = np.empty((N_CORES, BL, NA), dtype=np.float32)
    for c in range(N_CORES):
        logit[c][perms[c]] = res.results[c]["logit"]
    part = np.stack([res.results[c]["partials"][0] for c in range(N_CORES)])
    num = part[:, 0:3].sum(axis=0)
    den = part[:, 4:7].sum(axis=0)
    loss = np.float32(0.0)
    for g in range(3):
        loss = np.float32(loss + (-num[g]) / max(den[g], np.float32(1.0)))
    return logit.reshape(B, NA), np.float32(loss)


# revision 7
# speedup vs baseline: 1.0147x; 1.0147x over previous
"""Trainium2 Bass kernel for masked-mean-pooling + per-attribute softmax/loss.

Data-parallel over the batch: 8 NeuronCores x 2048 users each.

Gather strategy: dma_gather is the only high-rate gather primitive but takes
int16 indices, so the table is scattered (fixed pseudo-random permutation)
into 4 equal chunks of 25024 rows. Per user the indices are deduplicated and
carried with multiplicity weights; the randomized chunk mapping keeps the
per-user per-chunk distinct counts near Binomial(d, 1/4), so compile-time
per-tile window widths (users sorted by their max per-chunk count) fit with
high probability. Unused window slots point at a zero table row with weight
zero. Three window profiles (tight/medium/safe) are compiled on demand and
picked per input by an exact host-side fit check, so the kernel is never
wrong, only slower on unusual inputs.

Each 1024-index gather lands in its own SBUF tile (single-packet mode, 4
SWDGE queues round-robin); a DVE multiply applies the multiplicity weights
into a per-tile accumulation buffer, one strided reduce sums the history
dim, the mask-length column is appended, PE transposes [128,65]->[65,128]
and a matmul against the host-packed [65,16] weights (bias folded in via the
length column) produces the logits' pre-activations. Epilogue: scale by
1/len, per-attribute softmax + log-softmax, per-core loss partials reduced
across partitions with a ones-matmul; the host sums partials into the
scalar loss.
"""

import os
import sys

if "/opt/trn_rl_repo" not in sys.path:
    sys.path.insert(0, "/opt/trn_rl_repo")

import numpy as np

N_CORES = 8
B, L, V, D = 16384, 50, 100000, 64
BL = B // N_CORES          # 2048 users per core
P = 128                    # partitions
NT = BL // P               # 16 tiles per core
ATTR = (2, 6, 8)
NA = 16

NCHUNK = 4
CHN = 25024                # rows per chunk (int16-addressable)
DUMMY = CHN                # local index of the zero row in each chunk block
VDEV = NCHUNK * (CHN + 1)

# fixed pseudo-random table permutation: original row -> (chunk, local)
_tp_pos = np.random.default_rng(12345).permutation(NCHUNK * CHN)
TP_CHUNK = (_tp_pos // CHN).astype(np.int32)       # [100096]
TP_LOCAL = (_tp_pos % CHN).astype(np.int32)

# per-tile window widths (applied to each of the 4 chunks)
PROFILE_T = (28, 19, 18, 17, 16, 15, 14, 13, 12, 12, 11, 10, 8, 8, 7, 5)
PROFILE_M = (29, 22, 21, 21, 20, 20, 20, 19, 19, 19, 18, 18, 18, 18, 17, 17)
PROFILE_S = (50,) * 16

_CACHE = {}


def _build_nc(profile):
    import concourse.bass as bass
    import concourse.tile as tile
    from concourse import mybir
    from concourse.bacc import Bacc
    from concourse.masks import make_identity

    f32 = mybir.dt.float32
    i16 = mybir.dt.int16
    Alu = mybir.AluOpType
    Ax = mybir.AxisListType
    Act = mybir.ActivationFunctionType

    tot_cols = [NCHUNK * w for w in profile]
    tot = sum(tot_cols)
    idx_free = 8 * tot

    nc = Bacc(None, target_bir_lowering=False, num_swdge_queues=4)

    idx_d = nc.dram_tensor("idx", [16, idx_free], i16, kind="ExternalInput")
    wg_d = nc.dram_tensor("wg", [P, tot], f32, kind="ExternalInput")
    mask_d = nc.dram_tensor("mask", [BL, L], f32, kind="ExternalInput")
    y_d = nc.dram_tensor("y", [BL, NA], f32, kind="ExternalInput")
    ob_d = nc.dram_tensor("ob", [BL, NA], f32, kind="ExternalInput")
    emb_d = nc.dram_tensor("emb", [VDEV, D], f32, kind="ExternalInput")
    wbT_d = nc.dram_tensor("wbT", [D + 1, NA], f32, kind="ExternalInput")
    logit_d = nc.dram_tensor("logit", [BL, NA], f32, kind="ExternalOutput")
    part_d = nc.dram_tensor("partials", [1, 8], f32, kind="ExternalOutput")

    acc_bufs = 1 if max(profile) >= 40 else 3

    with tile.TileContext(nc) as tc:
        with (
            tc.tile_pool(name="const", bufs=1) as constp,
            tc.tile_pool(name="gsub", bufs=12) as gsubp,
            tc.tile_pool(name="acc", bufs=acc_bufs) as accp,
            tc.tile_pool(name="work", bufs=3) as workp,
            tc.tile_pool(name="epil", bufs=1) as epil,
            tc.tile_pool(name="pst", bufs=2, space="PSUM") as pst,
            tc.tile_pool(name="psw", bufs=1, space="PSUM") as psw,
        ):
            identity = constp.tile([P, P], f32)
            make_identity(nc, identity[:])

            wbT = constp.tile([D + 1, NA], f32)
            nc.sync.dma_start(out=wbT[:], in_=wbT_d[:])

            idx_sb = constp.tile([P, idx_free], i16)
            # split the replicated load so the first gathers start early
            splits = [0, 8 * tot_cols[0], idx_free]
            for a, b in zip(splits[:-1], splits[1:]):
                src = bass.AP(tensor=idx_d[:].tensor, offset=a,
                              ap=[[0, 8], [idx_free, 16], [1, b - a]])
                nc.sync.dma_start(out=idx_sb[:, a:b], in_=src)

            wg_sb = constp.tile([P, tot, 1], f32)
            nc.sync.dma_start(out=wg_sb[:], in_=wg_d[:].rearrange("p t -> p t ()"))

            mask_sb = constp.tile([P, NT, L], f32)
            nc.sync.dma_start(
                out=mask_sb[:], in_=mask_d.rearrange("(t p) l -> p t l", p=P)
            )
            y_sb = constp.tile([P, NT, NA], f32)
            nc.sync.dma_start(out=y_sb[:], in_=y_d.rearrange("(t p) j -> p t j", p=P))
            ob_sb = constp.tile([P, NT, NA], f32)
            nc.sync.dma_start(out=ob_sb[:], in_=ob_d.rearrange("(t p) j -> p t j", p=P))

            ones = constp.tile([P, 1], f32)
            nc.vector.memset(ones[:], 1.0)

            lens = epil.tile([P, NT, 1], f32)
            rlen = epil.tile([P, NT, 1], f32)
            W_ps = psw.tile([P, NT, NA], f32)

            qn = 0
            off = 0   # column offset into the global stream
            for t in range(NT):
                wt = profile[t]
                tcols = tot_cols[t]
                embacc = accp.tile([P, tcols, D], f32, tag="acc")
                cbase = 0
                for k in range(NCHUNK):
                    emb_view = emb_d[k * (CHN + 1):(k + 1) * (CHN + 1), :]
                    c0 = 0
                    while c0 < wt:
                        w = min(8, wt - c0)
                        ni = w * P
                        col = off + cbase + c0
                        gsub = gsubp.tile([P, 8, D], f32, tag="g")
                        nc.gpsimd.dma_gather(
                            out_ap=gsub[:, 0:w, :],
                            in_ap=emb_view,
                            idxs_ap=idx_sb[:, col * 8: (col + w) * 8],
                            num_idxs=ni,
                            num_idxs_reg=ni,
                            elem_size=D,
                            single_packet=True,
                            queue_num=qn,
                        )
                        qn = (qn + 1) % 4
                        nc.vector.tensor_tensor(
                            out=embacc[:, cbase + c0: cbase + c0 + w, :],
                            in0=gsub[:, 0:w, :],
                            in1=wg_sb[:, col: col + w, :].to_broadcast([P, w, D]),
                            op=Alu.mult,
                        )
                        c0 += w
                    cbase += wt
                off += tcols

                s65 = workp.tile([P, D + 1], f32)
                nc.vector.tensor_reduce(
                    out=s65[:, 0:D],
                    in_=embacc[:].rearrange("p c d -> p d c"),
                    axis=Ax.X,
                    op=Alu.add,
                )
                nc.vector.tensor_reduce(
                    out=lens[:, t, :], in_=mask_sb[:, t, :], axis=Ax.X, op=Alu.add
                )
                nc.vector.tensor_copy(out=s65[:, D: D + 1], in_=lens[:, t, :])
                tp = pst.tile([D + 1, P], f32)
                nc.tensor.transpose(out=tp[:], in_=s65[:], identity=identity[:])
                urT = workp.tile([D + 1, P], f32)
                nc.scalar.copy(out=urT[:], in_=tp[:])
                nc.tensor.matmul(W_ps[:, t, :], urT[:], wbT[:], start=True, stop=True)

            # ---- epilogue over all 2048 users: [P, NT, 16] ----
            nc.vector.reciprocal(out=rlen[:], in_=lens[:])
            W = epil.tile([P, NT, NA], f32)
            nc.vector.tensor_tensor(
                out=W[:], in0=W_ps[:], in1=rlen[:].to_broadcast([P, NT, NA]),
                op=Alu.mult,
            )

            S = epil.tile([P, NT, NA], f32)
            E = epil.tile([P, NT, NA], f32)
            LG = epil.tile([P, NT, NA], f32)
            yob = epil.tile([P, NT, NA], f32)
            nc.vector.tensor_tensor(out=yob[:], in0=y_sb[:], in1=ob_sb[:], op=Alu.mult)
            part = epil.tile([P, 8], f32)
            nc.vector.memset(part[:], 0.0)

            s = 0
            for g, w in enumerate(ATTR):
                sl = slice(s, s + w)
                mx = epil.tile([P, NT, 1], f32, tag=f"mx{g}")
                nc.vector.tensor_reduce(
                    out=mx[:], in_=W[:, :, sl], axis=Ax.X, op=Alu.max
                )
                nc.vector.tensor_tensor(
                    out=S[:, :, sl], in0=W[:, :, sl],
                    in1=mx[:].to_broadcast([P, NT, w]), op=Alu.subtract,
                )
                s += w

            nc.scalar.activation(out=E[:], in_=S[:], func=Act.Exp)

            s = 0
            for g, w in enumerate(ATTR):
                sl = slice(s, s + w)
                se = epil.tile([P, NT, 1], f32, tag=f"se{g}")
                nc.vector.tensor_reduce(
                    out=se[:], in_=E[:, :, sl], axis=Ax.X, op=Alu.add
                )
                rse = epil.tile([P, NT, 1], f32, tag=f"rse{g}")
                nc.vector.reciprocal(out=rse[:], in_=se[:])
                nc.vector.tensor_tensor(
                    out=LG[:, :, sl], in0=E[:, :, sl],
                    in1=rse[:].to_broadcast([P, NT, w]), op=Alu.mult,
                )
                lse = epil.tile([P, NT, 1], f32, tag=f"lse{g}")
                nc.scalar.activation(out=lse[:], in_=se[:], func=Act.Ln)
                nc.vector.tensor_tensor(
                    out=S[:, :, sl], in0=S[:, :, sl],
                    in1=lse[:].to_broadcast([P, NT, w]), op=Alu.subtract,
                )
                C = epil.tile([P, NT, w], f32, tag=f"C{g}")
                nc.vector.tensor_tensor(
                    out=C[:], in0=yob[:, :, sl], in1=S[:, :, sl], op=Alu.mult
                )
                nc.vector.tensor_reduce(
                    out=part[:, g: g + 1], in_=C[:], axis=Ax.XY, op=Alu.add
                )
                om = epil.tile([P, NT, 1], f32, tag=f"om{g}")
                nc.vector.tensor_reduce(
                    out=om[:], in_=ob_sb[:, :, sl], axis=Ax.X, op=Alu.max
                )
                nc.vector.tensor_reduce(
                    out=part[:, 4 + g: 5 + g], in_=om[:], axis=Ax.XY, op=Alu.add
                )
                s += w

            red = pst.tile([1, 8], f32, tag="red")
            nc.tensor.matmul(red[:], ones[:], part[:], start=True, stop=True)
            out_sb = epil.tile([1, 8], f32)
            nc.scalar.copy(out=out_sb[:], in_=red[:])
            nc.sync.dma_start(out=part_d[:], in_=out_sb[:])
            nc.sync.dma_start(
                out=logit_d.rearrange("(t p) j -> p t j", p=P), in_=LG[:]
            )

    nc.compile()
    return nc


def _get_nc(profile):
    if profile not in _CACHE:
        _CACHE[profile] = _build_nc(profile)
    return _CACHE[profile]


def _analyze_core(xc):
    """Dedup + chunk stats for one core. Returns dict of arrays."""
    xs = np.sort(xc, axis=1)                      # [BL, L]
    fo = np.concatenate(
        [np.ones((BL, 1), bool), xs[:, 1:] != xs[:, :-1]], axis=1
    )
    wgt = (xs[:, :, None] == xs[:, None, :]).sum(axis=2).astype(np.float32)
    ch = TP_CHUNK[xs]
    loc = TP_LOCAL[xs]
    n = np.stack([((ch == k) & fo).sum(axis=1) for k in range(NCHUNK)], axis=1)
    return dict(xs=xs, fo=fo, wgt=wgt, ch=ch, loc=loc, n=n)


def _fits(an, profile):
    nmax_sorted = np.sort(an["n"].max(axis=1))[::-1]
    wt = np.repeat(np.asarray(profile), P)
    return (nmax_sorted <= wt).all()


def _pack_core(an, profile):
    """Build slot permutation, idx stream and weight stream for one core."""
    nmax = an["n"].max(axis=1)
    order = np.argsort(-nmax, kind="stable")      # slot s -> original user
    fo, ch, loc, wgt = an["fo"], an["ch"], an["loc"], an["wgt"]

    idx_blocks = []
    wg_blocks = []
    for t in range(NT):
        wt = profile[t]
        users = order[t * P:(t + 1) * P]
        kt = ch[users]
        lt = loc[users]
        ft = fo[users]
        gt = wgt[users]
        for kk in range(NCHUNK):
            m = ft & (kt == kk)
            cc = np.cumsum(m, axis=1) - 1
            A = np.full((P, wt), DUMMY, dtype=np.int16)
            WG = np.zeros((P, wt), dtype=np.float32)
            rows, cols = np.where(m)
            A[rows, cc[rows, cols]] = lt[rows, cols].astype(np.int16)
            WG[rows, cc[rows, cols]] = gt[rows, cols]
            flat = A.T.ravel()                    # stream pos i = c*128 + p
            idx_blocks.append(flat.reshape(-1, 16).T)
            wg_blocks.append(WG)
    idx_wrapped = np.ascontiguousarray(np.concatenate(idx_blocks, axis=1))
    wg_host = np.ascontiguousarray(np.concatenate(wg_blocks, axis=1))
    return order, idx_wrapped, wg_host


def _install_trace_shim():
    import types

    import antenv

    if "antenv.axon_hooks" in sys.modules:
        return
    mod = types.ModuleType("antenv.axon_hooks")
    hook = [None]
    mod.set_axon_ntff_profile_hook = lambda h: hook.__setitem__(0, h)
    mod.get_axon_ntff_profile_hook = lambda: hook[0]
    sys.modules["antenv.axon_hooks"] = mod
    antenv.axon_hooks = mod
    if "/root/.axon_site" not in sys.path:
        sys.path.insert(0, "/root/.axon_site")
    try:
        from trn_agent_boot.trn_boot import _ntff_profile_via_ctypes

        mod.set_axon_ntff_profile_hook(
            _ntff_profile_via_ctypes("/opt/axon/libaxon_pjrt.so")
        )
    except Exception:
        pass


def kernel(**inputs):
    from concourse.bass_utils import run_bass_kernel_spmd

    x = np.clip(
        np.asarray(inputs["x"]).astype(np.int32), 0, V - 1
    ).reshape(N_CORES, BL, L)
    mask = np.asarray(inputs["x_mask"]).astype(np.float32).reshape(N_CORES, BL, L)
    y = np.asarray(inputs["y"]).astype(np.float32).reshape(N_CORES, BL, NA)
    ob = np.asarray(inputs["ob"]).astype(np.float32).reshape(N_CORES, BL, NA)
    emb = np.asarray(inputs["item_emb"], dtype=np.float32)

    # permuted device table: 4 blocks of CHN+1 rows, last row of each is zero
    emb_dev = np.zeros((VDEV, D), dtype=np.float32)
    rows = np.arange(V)
    emb_dev[TP_CHUNK[:V] * (CHN + 1) + TP_LOCAL[:V]] = emb[rows]
    emb_dev = np.ascontiguousarray(emb_dev)

    wcat = np.concatenate(
        [np.asarray(inputs[f"w{i}"], dtype=np.float32) for i in range(3)], axis=0
    )
    bcat = np.concatenate(
        [np.asarray(inputs[f"b{i}"], dtype=np.float32) for i in range(3)], axis=0
    )
    wbT = np.ascontiguousarray(
        np.concatenate([wcat.T, bcat[None, :]], axis=0).astype(np.float32)
    )

    analyses = [_analyze_core(x[c]) for c in range(N_CORES)]
    profile = None
    for prof in (PROFILE_T, PROFILE_M, PROFILE_S):
        if all(_fits(an, prof) for an in analyses):
            profile = prof
            break
    assert profile is not None  # PROFILE_S always fits (counts <= 50)

    in_maps = []
    perms = []
    for c in range(N_CORES):
        perm, idxw, wgh = _pack_core(analyses[c], profile)
        perms.append(perm)
        in_maps.append({
            "idx": idxw,
            "wg": wgh,
            "mask": np.ascontiguousarray(mask[c][perm]),
            "y": np.ascontiguousarray(y[c][perm]),
            "ob": np.ascontiguousarray(ob[c][perm]),
            "emb": emb_dev,
            "wbT": wbT,
        })

    nc = _get_nc(profile)
    trace = os.environ.get("KERNEL_TRACE") == "1"
    if trace:
        _install_trace_shim()
    res = run_bass_kernel_spmd(
        nc, in_maps, core_ids=list(range(N_CORES)), trace=trace
    )
    if trace:
        _CACHE["exec_time_ns"] = res.exec_time_ns
        _CACHE["profile_json"] = res.profile_json

    logit = np.empty((N_CORES, BL, NA), dtype=np.float32)
    for c in range(N_CORES):
        logit[c][perms[c]] = res.results[c]["logit"]
    part = np.stack([res.results[c]["partials"][0] for c in range(N_CORES)])
    num = part[:, 0:3].sum(axis=0)
    den = part[:, 4:7].sum(axis=0)
    loss = np.float32(0.0)
    for g in range(3):
        loss = np.float32(loss + (-num[g]) / max(den[g], np.float32(1.0)))
    return logit.reshape(B, NA), np.float32(loss)


# revision 9
# speedup vs baseline: 1.3047x; 1.2859x over previous
"""Trainium2 Bass kernel for masked-mean-pooling + per-attribute softmax/loss.

Data-parallel over the batch: 8 NeuronCores x 2048 users each.

Gather strategy: dma_gather is the only high-rate gather primitive but takes
int16 indices, so the table is scattered (fixed pseudo-random permutation)
into 4 equal chunks of 25024 rows. Per user the indices are deduplicated and
carried with multiplicity weights; the randomized chunk mapping keeps the
per-user per-chunk distinct counts near Binomial(d, 1/4), so compile-time
per-tile window widths (users sorted by their max per-chunk count) fit with
high probability. Unused window slots point at a zero table row with weight
zero. Three window profiles (tight/medium/safe) are compiled on demand and
picked per input by an exact host-side fit check, so the kernel is never
wrong, only slower on unusual inputs.

Each 1024-index gather lands in its own SBUF tile (single-packet mode, 4
SWDGE queues round-robin); a DVE multiply applies the multiplicity weights
into a per-tile accumulation buffer, one strided reduce sums the history
dim, the mask-length column is appended, PE transposes [128,65]->[65,128]
and a matmul against the host-packed [65,16] weights (bias folded in via the
length column) produces the logits' pre-activations. Epilogue: scale by
1/len, per-attribute softmax + log-softmax, per-core loss partials reduced
across partitions with a ones-matmul; the host sums partials into the
scalar loss.
"""

import os
import sys

if "/opt/trn_rl_repo" not in sys.path:
    sys.path.insert(0, "/opt/trn_rl_repo")

import numpy as np

N_CORES = 8
B, L, V, D = 16384, 50, 100000, 64
BL = B // N_CORES          # 2048 users per core
P = 128                    # partitions
NT = BL // P               # 16 tiles per core
ATTR = (2, 6, 8)
NA = 16

NCHUNK = 4
CHN = 25024                # rows per chunk (int16-addressable)
DUMMY = CHN                # local index of the zero row in each chunk block
VDEV = NCHUNK * (CHN + 1)

# fixed pseudo-random table permutation: original row -> (chunk, local)
_tp_pos = np.random.default_rng(12345).permutation(NCHUNK * CHN)
TP_CHUNK = (_tp_pos // CHN).astype(np.int32)       # [100096]
TP_LOCAL = (_tp_pos % CHN).astype(np.int32)

# per-tile window widths (applied to each of the 4 chunks)
PROFILE_T = (27, 17, 16, 15, 14, 13, 12, 11, 10, 10, 9, 8, 6, 6, 5, 3)
PROFILE_M = (29, 22, 21, 21, 20, 20, 20, 19, 19, 19, 18, 18, 18, 18, 17, 17)
PROFILE_S = (50,) * 16

_CACHE = {}


def _build_nc(profile):
    import concourse.bass as bass
    import concourse.tile as tile
    from concourse import mybir
    from concourse.bacc import Bacc
    from concourse.masks import make_identity

    f32 = mybir.dt.float32
    i16 = mybir.dt.int16
    Alu = mybir.AluOpType
    Ax = mybir.AxisListType
    Act = mybir.ActivationFunctionType

    tot_cols = [NCHUNK * w for w in profile]
    tot = sum(tot_cols)
    idx_free = 8 * tot

    nc = Bacc(None, target_bir_lowering=False, num_swdge_queues=4)

    idx_d = nc.dram_tensor("idx", [16, idx_free], i16, kind="ExternalInput")
    wg_d = nc.dram_tensor("wg", [P, tot], f32, kind="ExternalInput")
    mask_d = nc.dram_tensor("mask", [BL, L], f32, kind="ExternalInput")
    y_d = nc.dram_tensor("y", [BL, NA], f32, kind="ExternalInput")
    ob_d = nc.dram_tensor("ob", [BL, NA], f32, kind="ExternalInput")
    emb_d = nc.dram_tensor("emb", [VDEV, D], f32, kind="ExternalInput")
    wbT_d = nc.dram_tensor("wbT", [D + 1, NA], f32, kind="ExternalInput")
    logit_d = nc.dram_tensor("logit", [BL, NA], f32, kind="ExternalOutput")
    part_d = nc.dram_tensor("partials", [1, 8], f32, kind="ExternalOutput")

    acc_bufs = 1 if max(profile) >= 40 else 3

    with tile.TileContext(nc) as tc:
        with (
            tc.tile_pool(name="const", bufs=1) as constp,
            tc.tile_pool(name="gsub", bufs=16) as gsubp,
            tc.tile_pool(name="acc", bufs=acc_bufs) as accp,
            tc.tile_pool(name="work", bufs=3) as workp,
            tc.tile_pool(name="epil", bufs=1) as epil,
            tc.tile_pool(name="pst", bufs=2, space="PSUM") as pst,
            tc.tile_pool(name="psw", bufs=1, space="PSUM") as psw,
        ):
            identity = constp.tile([P, P], f32)
            make_identity(nc, identity[:])

            wbT = constp.tile([D + 1, NA], f32)
            nc.sync.dma_start(out=wbT[:], in_=wbT_d[:])

            idx_sb = constp.tile([P, idx_free], i16)
            # split the replicated load so the first gathers start early
            splits = [0, 8 * tot_cols[0], idx_free]
            for a, b in zip(splits[:-1], splits[1:]):
                src = bass.AP(tensor=idx_d[:].tensor, offset=a,
                              ap=[[0, 8], [idx_free, 16], [1, b - a]])
                nc.sync.dma_start(out=idx_sb[:, a:b], in_=src)

            wg_sb = constp.tile([P, tot, 1], f32)
            nc.sync.dma_start(out=wg_sb[:], in_=wg_d[:].rearrange("p t -> p t ()"))

            mask_sb = constp.tile([P, NT, L], f32)
            nc.sync.dma_start(
                out=mask_sb[:], in_=mask_d.rearrange("(t p) l -> p t l", p=P)
            )
            y_sb = constp.tile([P, NT, NA], f32)
            nc.sync.dma_start(out=y_sb[:], in_=y_d.rearrange("(t p) j -> p t j", p=P))
            ob_sb = constp.tile([P, NT, NA], f32)
            nc.sync.dma_start(out=ob_sb[:], in_=ob_d.rearrange("(t p) j -> p t j", p=P))

            ones = constp.tile([P, 1], f32)
            nc.vector.memset(ones[:], 1.0)

            lens = epil.tile([P, NT, 1], f32)
            rlen = epil.tile([P, NT, 1], f32)
            W_ps = psw.tile([P, NT, NA], f32)

            qn = 0
            off = 0   # column offset into the global stream
            for t in range(NT):
                wt = profile[t]
                tcols = tot_cols[t]
                embacc = accp.tile([P, tcols, D], f32, tag="acc")
                cbase = 0
                for k in range(NCHUNK):
                    emb_view = emb_d[k * (CHN + 1):(k + 1) * (CHN + 1), :]
                    c0 = 0
                    while c0 < wt:
                        w = min(8, wt - c0)
                        ni = w * P
                        col = off + cbase + c0
                        gsub = gsubp.tile([P, 8, D], f32, tag="g")
                        nc.gpsimd.dma_gather(
                            out_ap=gsub[:, 0:w, :],
                            in_ap=emb_view,
                            idxs_ap=idx_sb[:, col * 8: (col + w) * 8],
                            num_idxs=ni,
                            num_idxs_reg=ni,
                            elem_size=D,
                            single_packet=True,
                            queue_num=qn,
                        )
                        qn = (qn + 1) % 4
                        nc.vector.tensor_tensor(
                            out=embacc[:, cbase + c0: cbase + c0 + w, :],
                            in0=gsub[:, 0:w, :],
                            in1=wg_sb[:, col: col + w, :].to_broadcast([P, w, D]),
                            op=Alu.mult,
                        )
                        c0 += w
                    cbase += wt
                off += tcols

                s65 = workp.tile([P, D + 1], f32)
                nc.vector.tensor_reduce(
                    out=s65[:, 0:D],
                    in_=embacc[:].rearrange("p c d -> p d c"),
                    axis=Ax.X,
                    op=Alu.add,
                )
                nc.vector.tensor_reduce(
                    out=lens[:, t, :], in_=mask_sb[:, t, :], axis=Ax.X, op=Alu.add
                )
                nc.vector.tensor_copy(out=s65[:, D: D + 1], in_=lens[:, t, :])
                tp = pst.tile([D + 1, P], f32)
                nc.tensor.transpose(out=tp[:], in_=s65[:], identity=identity[:])
                urT = workp.tile([D + 1, P], f32)
                nc.scalar.copy(out=urT[:], in_=tp[:])
                nc.tensor.matmul(W_ps[:, t, :], urT[:], wbT[:], start=True, stop=True)

            # ---- epilogue over all 2048 users: [P, NT, 16] ----
            nc.vector.reciprocal(out=rlen[:], in_=lens[:])
            W = epil.tile([P, NT, NA], f32)
            nc.vector.tensor_tensor(
                out=W[:], in0=W_ps[:], in1=rlen[:].to_broadcast([P, NT, NA]),
                op=Alu.mult,
            )

            S = epil.tile([P, NT, NA], f32)
            E = epil.tile([P, NT, NA], f32)
            LG = epil.tile([P, NT, NA], f32)
            yob = epil.tile([P, NT, NA], f32)
            nc.vector.tensor_tensor(out=yob[:], in0=y_sb[:], in1=ob_sb[:], op=Alu.mult)
            part = epil.tile([P, 8], f32)
            nc.vector.memset(part[:], 0.0)

            s = 0
            for g, w in enumerate(ATTR):
                sl = slice(s, s + w)
                mx = epil.tile([P, NT, 1], f32, tag=f"mx{g}")
                nc.vector.tensor_reduce(
                    out=mx[:], in_=W[:, :, sl], axis=Ax.X, op=Alu.max
                )
                nc.vector.tensor_tensor(
                    out=S[:, :, sl], in0=W[:, :, sl],
                    in1=mx[:].to_broadcast([P, NT, w]), op=Alu.subtract,
                )
                s += w

            nc.scalar.activation(out=E[:], in_=S[:], func=Act.Exp)

            s = 0
            for g, w in enumerate(ATTR):
                sl = slice(s, s + w)
                se = epil.tile([P, NT, 1], f32, tag=f"se{g}")
                nc.vector.tensor_reduce(
                    out=se[:], in_=E[:, :, sl], axis=Ax.X, op=Alu.add
                )
                rse = epil.tile([P, NT, 1], f32, tag=f"rse{g}")
                nc.vector.reciprocal(out=rse[:], in_=se[:])
                nc.vector.tensor_tensor(
                    out=LG[:, :, sl], in0=E[:, :, sl],
                    in1=rse[:].to_broadcast([P, NT, w]), op=Alu.mult,
                )
                lse = epil.tile([P, NT, 1], f32, tag=f"lse{g}")
                nc.scalar.activation(out=lse[:], in_=se[:], func=Act.Ln)
                nc.vector.tensor_tensor(
                    out=S[:, :, sl], in0=S[:, :, sl],
                    in1=lse[:].to_broadcast([P, NT, w]), op=Alu.subtract,
                )
                C = epil.tile([P, NT, w], f32, tag=f"C{g}")
                nc.vector.tensor_tensor(
                    out=C[:], in0=yob[:, :, sl], in1=S[:, :, sl], op=Alu.mult
                )
                nc.vector.tensor_reduce(
                    out=part[:, g: g + 1], in_=C[:], axis=Ax.XY, op=Alu.add
                )
                om = epil.tile([P, NT, 1], f32, tag=f"om{g}")
                nc.vector.tensor_reduce(
                    out=om[:], in_=ob_sb[:, :, sl], axis=Ax.X, op=Alu.max
                )
                nc.vector.tensor_reduce(
                    out=part[:, 4 + g: 5 + g], in_=om[:], axis=Ax.XY, op=Alu.add
                )
                s += w

            red = pst.tile([1, 8], f32, tag="red")
            nc.tensor.matmul(red[:], ones[:], part[:], start=True, stop=True)
            out_sb = epil.tile([1, 8], f32)
            nc.scalar.copy(out=out_sb[:], in_=red[:])
            nc.sync.dma_start(out=part_d[:], in_=out_sb[:])
            nc.sync.dma_start(
                out=logit_d.rearrange("(t p) j -> p t j", p=P), in_=LG[:]
            )

    nc.compile()
    return nc


def _get_nc(profile):
    if profile not in _CACHE:
        _CACHE[profile] = _build_nc(profile)
    return _CACHE[profile]


def _analyze_core(xc):
    """Dedup + chunk stats for one core. Returns dict of arrays."""
    xs = np.sort(xc, axis=1)                      # [BL, L]
    fo = np.concatenate(
        [np.ones((BL, 1), bool), xs[:, 1:] != xs[:, :-1]], axis=1
    )
    wgt = (xs[:, :, None] == xs[:, None, :]).sum(axis=2).astype(np.float32)
    ch = TP_CHUNK[xs]
    loc = TP_LOCAL[xs]
    n = np.stack([((ch == k) & fo).sum(axis=1) for k in range(NCHUNK)], axis=1)
    return dict(xs=xs, fo=fo, wgt=wgt, ch=ch, loc=loc, n=n)


def _fits(an, profile):
    nmax_sorted = np.sort(an["n"].max(axis=1))[::-1]
    wt = np.repeat(np.asarray(profile), P)
    return (nmax_sorted <= wt).all()


def _pack_core(an, profile):
    """Build slot permutation, idx stream and weight stream for one core."""
    nmax = an["n"].max(axis=1)
    order = np.argsort(-nmax, kind="stable")      # slot s -> original user
    fo, ch, loc, wgt = an["fo"], an["ch"], an["loc"], an["wgt"]

    idx_blocks = []
    wg_blocks = []
    for t in range(NT):
        wt = profile[t]
        users = order[t * P:(t + 1) * P]
        kt = ch[users]
        lt = loc[users]
        ft = fo[users]
        gt = wgt[users]
        for kk in range(NCHUNK):
            m = ft & (kt == kk)
            cc = np.cumsum(m, axis=1) - 1
            A = np.full((P, wt), DUMMY, dtype=np.int16)
            WG = np.zeros((P, wt), dtype=np.float32)
            rows, cols = np.where(m)
            A[rows, cc[rows, cols]] = lt[rows, cols].astype(np.int16)
            WG[rows, cc[rows, cols]] = gt[rows, cols]
            flat = A.T.ravel()                    # stream pos i = c*128 + p
            idx_blocks.append(flat.reshape(-1, 16).T)
            wg_blocks.append(WG)
    idx_wrapped = np.ascontiguousarray(np.concatenate(idx_blocks, axis=1))
    wg_host = np.ascontiguousarray(np.concatenate(wg_blocks, axis=1))
    return order, idx_wrapped, wg_host


def _install_trace_shim():
    import types

    import antenv

    if "antenv.axon_hooks" in sys.modules:
        return
    mod = types.ModuleType("antenv.axon_hooks")
    hook = [None]
    mod.set_axon_ntff_profile_hook = lambda h: hook.__setitem__(0, h)
    mod.get_axon_ntff_profile_hook = lambda: hook[0]
    sys.modules["antenv.axon_hooks"] = mod
    antenv.axon_hooks = mod
    if "/root/.axon_site" not in sys.path:
        sys.path.insert(0, "/root/.axon_site")
    try:
        from trn_agent_boot.trn_boot import _ntff_profile_via_ctypes

        mod.set_axon_ntff_profile_hook(
            _ntff_profile_via_ctypes("/opt/axon/libaxon_pjrt.so")
        )
    except Exception:
        pass


def kernel(**inputs):
    from concourse.bass_utils import run_bass_kernel_spmd

    x = np.clip(
        np.asarray(inputs["x"]).astype(np.int32), 0, V - 1
    ).reshape(N_CORES, BL, L)
    mask = np.asarray(inputs["x_mask"]).astype(np.float32).reshape(N_CORES, BL, L)
    y = np.asarray(inputs["y"]).astype(np.float32).reshape(N_CORES, BL, NA)
    ob = np.asarray(inputs["ob"]).astype(np.float32).reshape(N_CORES, BL, NA)
    emb = np.asarray(inputs["item_emb"], dtype=np.float32)

    # permuted device table: 4 blocks of CHN+1 rows, last row of each is zero
    emb_dev = np.zeros((VDEV, D), dtype=np.float32)
    rows = np.arange(V)
    emb_dev[TP_CHUNK[:V] * (CHN + 1) + TP_LOCAL[:V]] = emb[rows]
    emb_dev = np.ascontiguousarray(emb_dev)

    wcat = np.concatenate(
        [np.asarray(inputs[f"w{i}"], dtype=np.float32) for i in range(3)], axis=0
    )
    bcat = np.concatenate(
        [np.asarray(inputs[f"b{i}"], dtype=np.float32) for i in range(3)], axis=0
    )
    wbT = np.ascontiguousarray(
        np.concatenate([wcat.T, bcat[None, :]], axis=0).astype(np.float32)
    )

    analyses = [_analyze_core(x[c]) for c in range(N_CORES)]
    profile = None
    for prof in (PROFILE_T, PROFILE_M, PROFILE_S):
        if all(_fits(an, prof) for an in analyses):
            profile = prof
            break
    assert profile is not None  # PROFILE_S always fits (counts <= 50)

    in_maps = []
    perms = []
    for c in range(N_CORES):
        perm, idxw, wgh = _pack_core(analyses[c], profile)
        perms.append(perm)
        in_maps.append({
            "idx": idxw,
            "wg": wgh,
            "mask": np.ascontiguousarray(mask[c][perm]),
            "y": np.ascontiguousarray(y[c][perm]),
            "ob": np.ascontiguousarray(ob[c][perm]),
            "emb": emb_dev,
            "wbT": wbT,
        })

    nc = _get_nc(profile)
    trace = os.environ.get("KERNEL_TRACE") == "1"
    if trace:
        _install_trace_shim()
    res = run_bass_kernel_spmd(
        nc, in_maps, core_ids=list(range(N_CORES)), trace=trace
    )
    if trace:
        _CACHE["exec_time_ns"] = res.exec_time_ns
        _CACHE["profile_json"] = res.profile_json

    logit = np.empty((N_CORES, BL, NA), dtype=np.float32)
    for c in range(N_CORES):
        logit[c][perms[c]] = res.results[c]["logit"]
    part = np.stack([res.results[c]["partials"][0] for c in range(N_CORES)])
    num = part[:, 0:3].sum(axis=0)
    den = part[:, 4:7].sum(axis=0)
    loss = np.float32(0.0)
    for g in range(3):
        loss = np.float32(loss + (-num[g]) / max(den[g], np.float32(1.0)))
    return logit.reshape(B, NA), np.float32(loss)
